# revision 25
# baseline (speedup 1.0000x reference)
"""Trainium2 Bass kernel for the CustomLossFilter loss.

reference semantics (per row, fp32):
    cond = |inputs[:,4] - inputs[:,2]| < 0.1
    diff = where(cond, inputs[:,0] - inputs[:,4], inputs[:,0] - targets[:,0])
    out  = mean(|diff|)

Strategy: data-parallel over the 20M rows across 8 NeuronCores (2.5M rows
per core).  Inside a core, rows are mapped [128 partitions x 19531 rows]
with each partition owning a contiguous row range, so every DMA is a plain
contiguous 2D transfer.  Columns 0/2/4 are accessed with stride-5 APs in
SBUF.  The kernel is DMA-bound: the 16 DMA queues sustain ~27 GB/s each
(~430 GB/s aggregate), so the 60 MB/core floor is ~140 us.  Large tiles
(1056 rows/partition) keep descriptor overhead low; 6-deep input/target
rings keep the stream saturated while compute trails; per-tile work is
spread so no engine exceeds ~70% of the DMA cadence:

    GpSimd: d = in4-in2 (strided), and the previous tile's
            diff = c0-tt (scalar_tensor_tensor) -- emitted one tile late
            so it never blocks the next tile's front op in the in-order
            engine queue
    Vector: fused |d|<0.1 mask (one tensor_scalar: AND sign-clear then
            unsigned is_lt against 0.1f's bit pattern), then
            copy_predicated(tt, m, in4)
    Scalar: c0 = copy(in0) deinterleave, and the previous tile's
            ABS+row-accumulate into acc[:, t]

The 32 rows per core that don't fit the [128 x 19531] grid are summed on
the host (256 rows total across 8 cores).  Each core emits a
[128, n_tiles] tensor of per-tile partition sums; the host adds the
partials and divides by N.
"""

import numpy as np

import concourse.bacc as bacc
import concourse.mybir as mybir
from concourse import tile
from concourse.bass_utils import run_bass_kernel_spmd

N_TOTAL = 20_000_000
F = 5
N_CORES = 8
ROWS = N_TOTAL // N_CORES  # 2_500_000 rows per core
P = 128
W = 528  # rows per partition per tile
ERR_OK = 0.1

_ALU = mybir.AluOpType
_F32 = mybir.dt.float32
_U8 = mybir.dt.uint8
_ABS = mybir.ActivationFunctionType.Abs
_CPY = mybir.ActivationFunctionType.Copy


def _widths(rpp, w):
    """Full-width tiles plus a small final tile: subdividing the stream
    costs real DMA-queue time (descriptor overhead), so keep tiles big and
    only split the remainder so the post-stream compute drain runs on a
    small tile."""
    full, rem = divmod(rpp, w)
    widths = [w] * full
    if rem > 256:
        widths.extend([rem - 128, 128])
    elif rem:
        widths.append(rem)
    return widths


def _body(tc, inp, tgt, out, rows, w):
    nc = tc.nc
    rpp = rows // P  # rows per partition; rows - P*rpp handled on host

    widths = _widths(rpp, w)
    nt = len(widths)

    # [128, rpp*5] / [128, rpp] contiguous-per-partition views of DRAM
    in_main = inp[: P * rpp, :].rearrange("(p r) f -> p (r f)", p=P)
    tg_main = tgt[: P * rpp, :].rearrange("(p r) f -> p (r f)", p=P)

    with (
        tc.tile_pool(name="acc", bufs=1) as accpool,
        tc.tile_pool(name="inp", bufs=12) as inpool,
        tc.tile_pool(name="tgp", bufs=12) as tgpool,
        tc.tile_pool(name="dp", bufs=3) as dpool,
        tc.tile_pool(name="cp", bufs=3) as cpool,
        tc.tile_pool(name="fp", bufs=3) as fpool,
        tc.tile_pool(name="ap", bufs=2) as apool,
        tc.tile_pool(name="mp", bufs=3) as mpool,
    ):
        acc = accpool.tile([P, nt], _F32)

        off = 0
        for t, wt in enumerate(widths):
            ti = inpool.tile([P, w * F], _F32, tag="in")
            tt = tgpool.tile([P, w], _F32, tag="tg")
            nc.sync.dma_start(ti[:, : wt * F], in_main[:, off * F : (off + wt) * F])
            nc.scalar.dma_start(tt[:, :wt], tg_main[:, off : off + wt])

            in0 = ti[:, 0 : wt * F : F]
            in2 = ti[:, 2 : wt * F : F]
            in4 = ti[:, 4 : wt * F : F]

            d = dpool.tile([P, w], _F32, tag="d")
            c0 = cpool.tile([P, w], _F32, tag="c")
            m = mpool.tile([P, w], _U8, tag="m")
            diff = fpool.tile([P, w], _F32, tag="f")
            adiff = apool.tile([P, w], _F32, tag="a")  # write-only scratch
            du = d[:, :wt].bitcast(mybir.dt.uint32)
            nc.gpsimd.tensor_tensor(d[:, :wt], in4, in2, _ALU.subtract)
            nc.scalar.activation(c0[:, :wt], in0, _CPY)
            # exact |d| < 0.1f in int space: clear the sign bit, then
            # unsigned-compare against the bit pattern of 0.1f (positive
            # IEEE754 floats order like ints)
            nc.vector.tensor_scalar(du, du, 0x7FFFFFFF, None, _ALU.bitwise_and)
            nc.vector.tensor_scalar(m[:, :wt], du, 0x3DCCCCCD, None, _ALU.is_lt)
            nc.vector.copy_predicated(tt[:, :wt], m[:, :wt], in4)
            nc.vector.tensor_tensor(
                diff[:, :wt], c0[:, :wt], tt[:, :wt], _ALU.subtract
            )
            nc.scalar.activation(
                adiff[:, :wt], diff[:, :wt], _ABS, accum_out=acc[:, t : t + 1]
            )
            off += wt

        nc.sync.dma_start(out[:], acc[:])


def n_tiles(rows=ROWS, w=W):
    return len(_widths(rows // P, w))


def build_nc(rows=ROWS, w=W):
    nc = bacc.Bacc(
        "TRN2", target_bir_lowering=False, debug=False, num_devices=N_CORES
    )
    inp = nc.dram_tensor("inputs", [rows, F], _F32, kind="ExternalInput").ap()
    tgt = nc.dram_tensor("targets", [rows, 1], _F32, kind="ExternalInput").ap()
    out = nc.dram_tensor(
        "out", [P, n_tiles(rows, w)], _F32, kind="ExternalOutput"
    ).ap()
    with tile.TileContext(nc) as tc:
        _body(tc, inp, tgt, out, rows, w)
    nc.compile()
    return nc


_NC_CACHE = {}


def _get_nc():
    if "nc" not in _NC_CACHE:
        _NC_CACHE["nc"] = build_nc()
    return _NC_CACHE["nc"]


def _scrap_sum(inputs, targets):
    """|diff| sum over the rows each core's [128 x rpp] grid doesn't cover
    (32 rows per 2.5M-row shard; 256 rows total) -- done on the host."""
    rpp = ROWS // P
    main = P * rpp
    s = 0.0
    for i in range(N_CORES):
        lo = i * ROWS + main
        hi = (i + 1) * ROWS
        inp = inputs[lo:hi]
        tgt = targets[lo:hi, 0]
        cond = np.abs(inp[:, 4] - inp[:, 2]) < ERR_OK
        diff = np.where(cond, inp[:, 0] - inp[:, 4], inp[:, 0] - tgt)
        s += np.abs(diff).astype(np.float64).sum()
    return s


def run_sharded(inputs, targets, **spmd_kwargs):
    """Run the SPMD kernel; returns (per-core [128,nt] partials, results obj)."""
    nc = _get_nc()
    inputs = np.asarray(inputs, dtype=np.float32)
    targets = np.asarray(targets, dtype=np.float32)
    in_maps = [
        {
            "inputs": inputs[i * ROWS : (i + 1) * ROWS],
            "targets": targets[i * ROWS : (i + 1) * ROWS],
        }
        for i in range(N_CORES)
    ]
    res = run_bass_kernel_spmd(nc, in_maps, list(range(N_CORES)), **spmd_kwargs)
    partials = np.stack([r["out"] for r in res.results])  # [8, 128, n_tiles]
    return partials, res


def kernel(inputs, targets):
    inputs = np.asarray(inputs, dtype=np.float32)
    targets = np.asarray(targets, dtype=np.float32)
    partials, _ = run_sharded(inputs, targets)
    total = partials.astype(np.float64).sum() + _scrap_sum(inputs, targets)
    return np.asarray(total / N_TOTAL, dtype=np.float32)
